# revision 1
# baseline (speedup 1.0000x reference)
"""Trainium2 Bass kernel for nn_CapsuleNeuralNetworkV2 (8 cores, data-parallel).

Math (per sample, 8 capsule iterations then decoder):
  v = h.reshape(4, 196)
  q = v @ W1.T + b1 ; k = v @ W2.T + b2 ; u = v @ W3.T + b3
  scores[t,s] = q_t . k_s  ->  softmax over s -> h'_t = sum_s P[t,s] u_s
  dec = relu(h Wd1.T + bd1) Wd2.T + bd2 ; out = softmax(dec Wo.T + bo)

Key restructuring (host-side algebra):
  scores[t,s] = v_t . z_s + r_s  where  z_s = G v_s + c, r_s = a.v_s + d,
  G = W1.T W2, a = W2.T b1, c = W1.T b2, d = b1.b2.
  Since softmax rows sum to 1, u's bias b3 passes through the combine
  unchanged, so u = W3 v + b3 is computed with the bias fused in the matmul.

On-chip layout: batch-major h tile [128, 4, 197] (slot-stride 197, col 196
of each slot is a constant 1.0 used as the matmul bias row after the PE
transpose). All matmuls in float32r (full-rate fp32 on the PE for N>=256).
"""

import numpy as np

import concourse.bass as bass
import concourse.tile as tile
from concourse import bacc, mybir
from concourse.bass import ds
from concourse.bass_utils import run_bass_kernel_spmd
from concourse.masks import make_identity

FR = mybir.dt.float32r
BF = mybir.dt.bfloat16
F32 = mybir.dt.float32
AF = mybir.ActivationFunctionType
ALU = mybir.AluOpType

B = 32768
NCORES = 8
P = 128
T = 4
FV = 196
FEAT = 784
SLOT = FV + 2  # 198: slot data + ones column + zero-pad (keeps STT FD even-ish and fuses r)


def _ap(t, dims, offset_elems=0):
    """Hand-built AP over a tile's tensor: dims = [[step, count], ...] in elements."""
    a = t[:] if hasattr(t, "tile") or not isinstance(t, bass.AP) else t
    return bass.AP(tensor=a.tensor, offset=a.offset + offset_elems, ap=dims)


def build(nsub=4, ngroups=8):
    """One NeuronCore program processing nsub*ngroups*128 samples."""
    bpc = nsub * ngroups * P
    nc = bacc.Bacc("TRN2", target_bir_lowering=False, debug=False)

    x_d = nc.dram_tensor("x", [bpc, FEAT], FR, kind="ExternalInput")
    zu_d = nc.dram_tensor("zu_w", [P, 2, 394], FR, kind="ExternalInput")
    d1_d = nc.dram_tensor("dec1_w", [P, 8, FEAT], FR, kind="ExternalInput")
    d2_d = nc.dram_tensor("dec2_w", [P, 7, FEAT], FR, kind="ExternalInput")
    ow_d = nc.dram_tensor("out_w", [P, 7, 10], FR, kind="ExternalInput")
    out_d = nc.dram_tensor("out", [bpc, 10], F32, kind="ExternalOutput")

    with tile.TileContext(nc) as tc:
        consts = tc.alloc_tile_pool(name="consts", bufs=1)
        hp = tc.alloc_tile_pool(name="h", bufs=3)
        wk = tc.alloc_tile_pool(name="wk", bufs=3)
        wkd = tc.alloc_tile_pool(name="wkd", bufs=1)
        sm = tc.alloc_tile_pool(name="small", bufs=6)
        pp = tc.alloc_tile_pool(name="ps", bufs=2, space="PSUM")
        zup = pp
        dp = pp

        ident_f = consts.tile([P, P], F32)
        make_identity(nc, ident_f)
        ident = consts.tile([P, P], FR)
        nc.vector.tensor_copy(ident, ident_f)
        ones_c = consts.tile([P, 512], F32)
        nc.vector.memset(ones_c, 1.0)
        zu_w = consts.tile([P, 2, 394], FR)
        nc.sync.dma_start(out=zu_w, in_=zu_d[:, :, :])
        d1_w = consts.tile([P, 8, FEAT], FR)
        nc.sync.dma_start(out=d1_w, in_=d1_d[:, :, :])
        d2_w = consts.tile([P, 7, FEAT], FR)
        nc.sync.dma_start(out=d2_w, in_=d2_d[:, :, :])
        ow_w = consts.tile([P, 7, 10], FR)
        nc.sync.dma_start(out=ow_w, in_=ow_d[:, :, :])

        def capsule_iter(h_cur, h_nxt, j):
            """One capsule-attention iteration: h_nxt <- attn(h_cur)."""
            hb = wk.tile([P, T, SLOT], BF, tag="hb")
            nc.gpsimd.tensor_copy(hb, h_cur)
            # --- PE transposes: batch-major h -> feature-major V.T chunks ---
            vt1_ps = pp.tile([P, T, P], FR, tag="vt1ps")
            vt2_ps = pp.tile([69, T, P], FR, tag="vt2ps")
            for t in range(T):
                nc.tensor.transpose(vt1_ps[:, t, :], h_cur[:, t, 0:P], ident)
                # includes the ones column -> row 68 of the chunk is 1.0
                nc.tensor.transpose(vt2_ps[:, t, :], h_cur[:, t, P : P + 69], ident)
            vt1 = wk.tile([P, T, P], FR, tag="vt1")
            vt2 = wk.tile([69, T, P], FR, tag="vt2")
            nc.scalar.copy(vt1, vt1_ps)
            nc.scalar.copy(vt2, vt2_ps)

            u_sb = wk.tile([P, T, FV], FR, tag="usb")
            zb = wk.tile([P, T, SLOT], BF, tag="zb")
            dots = sm.tile([P, T, T], F32, tag="dots")
            scratch = sm.tile([P, SLOT], BF, tag="scr")

            for s in range(T):
                # z|r|u fused matmul for slot s: [128, 393] PSUM
                zu_ps = zup.tile([P, 394], F32, tag="zups")
                nc.tensor.matmul(zu_ps, vt1[:, s, :], zu_w[:, 0, :],
                                 start=True, stop=False)
                nc.tensor.matmul(zu_ps, vt2[:, s, :], zu_w[0:69, 1, :],
                                 start=False, stop=True)
                # evacuate u (fp32) and z|r|pad (bf16 for the dots)
                nc.scalar.copy(u_sb[:, s, :], zu_ps[:, 198:394])
                nc.scalar.copy(zb[:, s, :], zu_ps[:, 0:198])
                for t in range(T):
                    nc.vector.scalar_tensor_tensor(
                        out=scratch,
                        in0=hb[:, t, :],
                        scalar=1.0,
                        in1=zb[:, s, :],
                        op0=ALU.mult,
                        op1=ALU.mult,
                        accum_out=dots[:, t, s : s + 1],
                    )

            # softmax over s (no max subtraction; |scores| stays < 30)
            e = sm.tile([P, T, T], F32, tag="e")
            nc.scalar.activation(e, dots, AF.Exp)
            sums = sm.tile([P, T], F32, tag="sums")
            nc.vector.reduce_sum(sums, e, axis=mybir.AxisListType.X)
            rec = sm.tile([P, T], F32, tag="rec")
            nc.vector.reciprocal(rec, sums)
            probs = sm.tile([P, T, T], F32, tag="probs")
            nc.vector.scalar_tensor_tensor(
                out=probs, in0=e, scalar=1.0,
                in1=_ap(rec, [rec[:].ap[0], [1, T], [0, T]]),
                op0=ALU.mult, op1=ALU.mult,
            )

            # ones column for the next h
            nc.gpsimd.tensor_copy(h_nxt[:, :, 196:198], ones_c[:, 0 : 2 * T])
            # combine: h'_t = sum_s P[t,s] * u_s
            # chains t=0..2 on DVE (Pool seeds s=0); chain t=3 fully on Pool
            for t in range(3):
                nc.gpsimd.tensor_scalar_mul(
                    h_nxt[:, t, 0:FV], u_sb[:, 0, :], probs[:, t, 0:1]
                )
                for s in range(1, T):
                    nc.vector.scalar_tensor_tensor(
                        out=h_nxt[:, t, 0:FV],
                        in0=u_sb[:, s, :],
                        scalar=probs[:, t, s : s + 1],
                        in1=h_nxt[:, t, 0:FV],
                        op0=ALU.mult,
                        op1=ALU.add,
                    )
            pc_t = wk.tile([P, FV], F32, tag="pct")
            nc.gpsimd.tensor_scalar_mul(
                h_nxt[:, 3, 0:FV], u_sb[:, 0, :], probs[:, 3, 0:1]
            )
            for s in range(1, T):
                nc.gpsimd.tensor_scalar_mul(
                    pc_t, u_sb[:, s, :], probs[:, 3, s : s + 1]
                )
                nc.gpsimd.tensor_add(
                    h_nxt[:, 3, 0:FV], h_nxt[:, 3, 0:FV], pc_t
                )

        def decoder(hs, g):
            """Decoder over nsub tiles (N = nsub*128 wide matmuls)."""
            W = nsub * P
            # h.T chunks, slot-major: [128] x4 and [69] x4 (with ones row)
            ht1 = wkd.tile([P, T, W], FR, tag="ht1")
            ht2 = wkd.tile([69, T, W], FR, tag="ht2")
            for t in range(T):
                t1_ps = dp.tile([P, W], FR, tag="vt1ps")
                t2_ps = dp.tile([69, W], FR, tag="vt2ps")
                for j in range(nsub):
                    nc.tensor.transpose(
                        t1_ps[:, j * P : (j + 1) * P], hs[j][:, t, 0:P], ident
                    )
                    nc.tensor.transpose(
                        t2_ps[:, j * P : (j + 1) * P], hs[j][:, t, P : P + 69], ident
                    )
                nc.scalar.copy(ht1[:, t, :], t1_ps)
                nc.vector.tensor_copy(ht2[:, t, :], t2_ps)

            # dec1 = relu(Wd1 @ h.T + bd1), feature-major, 7 M-chunks
            d1a = wkd.tile([P, 6, W], FR, tag="d1a")
            d1b = wkd.tile([17, W], FR, tag="d1b")
            nc.vector.tensor_copy(d1b, ones_c[0:17, 0:W])
            for m in range(7):
                mw = min(P, FEAT - m * P)
                mp = dp.tile([P, W], F32, tag="zups")
                msl = slice(m * P, m * P + mw)
                for t in range(T):
                    nc.tensor.matmul(mp[0:mw, :], d1_w[:, t, msl], ht1[:, t, :],
                                     start=(t == 0), stop=False)
                for t in range(T):
                    nc.tensor.matmul(mp[0:mw, :], d1_w[0:69, 4 + t, msl],
                                     ht2[:, t, :], start=False, stop=(t == 3))
                if m < 6:
                    nc.scalar.activation(d1a[:, m, :], mp, AF.Relu)
                else:
                    nc.scalar.activation(d1b[0:16, :], mp[0:16, :], AF.Relu)

            # dec2 = Wd2 @ relu1 + bd2, feature-major
            d2a = wkd.tile([P, 6, W], FR, tag="d2a")
            d2b = wkd.tile([17, W], FR, tag="d2b")
            nc.vector.tensor_copy(d2b, ones_c[0:17, 0:W])
            for m in range(7):
                mw = min(P, FEAT - m * P)
                mp = dp.tile([P, W], F32, tag="zups")
                msl = slice(m * P, m * P + mw)
                for c in range(6):
                    nc.tensor.matmul(mp[0:mw, :], d2_w[:, c, msl], d1a[:, c, :],
                                     start=(c == 0), stop=False)
                nc.tensor.matmul(mp[0:mw, :], d2_w[0:17, 6, msl], d1b,
                                 start=False, stop=True)
                if m < 6:
                    nc.scalar.copy(d2a[:, m, :], mp)
                else:
                    nc.scalar.copy(d2b[0:16, :], mp[0:16, :])

            # logits + softmax per subtile
            for j in range(nsub):
                jsl = slice(j * P, (j + 1) * P)
                lg = dp.tile([P, 10], F32, tag="zups")
                for c in range(6):
                    nc.tensor.matmul(lg, d2a[:, c, jsl], ow_w[:, c, :],
                                     start=(c == 0), stop=False)
                nc.tensor.matmul(lg, d2b[:, jsl], ow_w[0:17, 6, :],
                                 start=False, stop=True)
                mx = sm.tile([P, 1], F32, tag="mx")
                nc.vector.reduce_max(mx, lg, axis=mybir.AxisListType.X)
                nmx = sm.tile([P, 1], F32, tag="nmx")
                nc.vector.tensor_scalar_mul(nmx, mx, -1.0)
                e10 = sm.tile([P, 10], F32, tag="e10")
                s10 = sm.tile([P, 1], F32, tag="s10")
                nc.scalar.activation(e10, lg, AF.Exp, bias=nmx, accum_out=s10)
                r10 = sm.tile([P, 1], F32, tag="r10")
                nc.vector.reciprocal(r10, s10)
                o10 = sm.tile([P, 10], F32, tag="o10")
                nc.vector.tensor_scalar_mul(o10, e10, r10)
                nc.sync.dma_start(
                    out=out_d[ds(g * (nsub * P) + j * P, P), :], in_=o10
                )

        def body(g):
            hs = []
            for j in range(nsub):
                h0 = hp.tile([P, T, SLOT], FR, tag=f"h{j}")
                nc.sync.dma_start(
                    out=h0[:, :, 0:FV],
                    in_=x_d[ds(g * (nsub * P) + j * P, P), :].rearrange(
                        "p (t f) -> p t f", t=T
                    ),
                )
                nc.gpsimd.tensor_copy(h0[:, :, 196:198], ones_c[:, 0 : 2 * T])
                hs.append(h0)
            for it in range(8):
                for j in range(nsub):
                    h_nxt = hp.tile([P, T, SLOT], FR, tag=f"h{j}")
                    capsule_iter(hs[j], h_nxt, j)
                    hs[j] = h_nxt
            decoder(hs, g)

        if ngroups == 1:
            body(0)
        else:
            with tc.For_i(0, ngroups, 1) as g:
                body(g)
        for _pool in (pp, sm, wkd, wk, hp, consts):
            _pool.release()

    nc.compile()
    return nc


def pack_weights(W1, b1, W2, b2, W3, b3, Wd1, bd1, Wd2, bd2, Wo, bo):
    f64 = np.float64
    W1, b1, W2, b2, W3, b3 = (np.asarray(t, f64) for t in (W1, b1, W2, b2, W3, b3))
    G = W1.T @ W2
    a = W2.T @ b1
    c = W1.T @ b2
    d = float(b1 @ b2)

    zu = np.zeros((P, 2, 394), np.float32)
    full = np.zeros((197, 394), f64)
    full[:196, :196] = G.T
    full[:196, 196] = a
    full[:196, 198:] = W3.T
    full[196, :196] = c
    full[196, 196] = d
    full[196, 198:] = b3
    zu[:, 0, :] = full[0:128]
    zu[0:69, 1, :] = full[128:197]

    d1 = np.zeros((P, 8, FEAT), np.float32)
    W1T = np.asarray(Wd1, f64).T  # [784 f_in, 784 j]
    for t in range(T):
        d1[:, t, :] = W1T[t * FV : t * FV + P, :]
        d1[0:68, 4 + t, :] = W1T[t * FV + P : (t + 1) * FV, :]
    d1[68, 4, :] = np.asarray(bd1, f64)

    d2 = np.zeros((P, 7, FEAT), np.float32)
    W2T = np.asarray(Wd2, f64).T
    for cidx in range(6):
        d2[:, cidx, :] = W2T[cidx * P : (cidx + 1) * P, :]
    d2[0:16, 6, :] = W2T[768:784, :]
    d2[16, 6, :] = np.asarray(bd2, f64)

    ow = np.zeros((P, 7, 10), np.float32)
    WoT = np.asarray(Wo, f64).T
    for cidx in range(6):
        ow[:, cidx, :] = WoT[cidx * P : (cidx + 1) * P, :]
    ow[0:16, 6, :] = WoT[768:784, :]
    ow[16, 6, :] = np.asarray(bo, f64)
    return zu, d1, d2, ow


_NC_CACHE = {}


def kernel(**inputs):
    x = np.ascontiguousarray(np.asarray(inputs["x"], np.float32))
    zu, d1, d2, ow = pack_weights(
        inputs["W1"], inputs["b1"], inputs["W2"], inputs["b2"], inputs["W3"],
        inputs["b3"], inputs["Wd1"], inputs["bd1"], inputs["Wd2"],
        inputs["bd2"], inputs["Wo"], inputs["bo"],
    )
    if "nc" not in _NC_CACHE:
        _NC_CACHE["nc"] = build(4, 8)
    nc = _NC_CACHE["nc"]
    bpc = B // NCORES
    in_maps = [
        {
            "x": x[c * bpc : (c + 1) * bpc],
            "zu_w": zu,
            "dec1_w": d1,
            "dec2_w": d2,
            "out_w": ow,
        }
        for c in range(NCORES)
    ]
    res = run_bass_kernel_spmd(nc, in_maps, core_ids=list(range(NCORES)))
    return np.concatenate([res.results[c]["out"] for c in range(NCORES)], axis=0)

